# revision 10
# baseline (speedup 1.0000x reference)
"""Trainium2 Bass kernel for BetaGNN message passing (8 NeuronCores).

Strategy:
  - Node rows sharded 8 ways (12800 padded rows per core, 100 blocks of 128).
  - H = relu(X @ W_in.T + b_in) computed per-shard on device (bf16), AllGather.
  - SpMM (A @ H): edges partitioned by destination row on host, grouped by
    (dest block, source bucket) and padded to fixed capacities. Each run is
    one dma_gather (int16 local indices into a <=32768-row bucket of the
    all-gathered H), then segment-summed via one-hot matmuls on the PE
    (M[e,d] = (iota_d == dloc_e) * val_e; PSUM accumulates over edge tiles).
  - AH AllGather, then SpMM2 fused with the dense head; logits per shard;
    host concatenates and trims padding.
"""

import sys
import types
from contextlib import ExitStack

import numpy as np

# ---------------------------------------------------------------- ntff hook
def _install_ntff_hook():
    """The image's antenv lacks axon_hooks; synthesize it so trace=True works."""
    if "antenv.axon_hooks" in sys.modules:
        return
    try:
        import antenv  # noqa: F401
    except ImportError:
        return
    mod = types.ModuleType("antenv.axon_hooks")
    mod._hook = None

    def set_axon_ntff_profile_hook(h):
        mod._hook = h

    def get_axon_ntff_profile_hook():
        return mod._hook

    mod.set_axon_ntff_profile_hook = set_axon_ntff_profile_hook
    mod.get_axon_ntff_profile_hook = get_axon_ntff_profile_hook
    sys.modules["antenv.axon_hooks"] = mod
    try:
        from trn_agent_boot.trn_boot import _ntff_profile_via_ctypes

        set_axon_ntff_profile_hook(
            _ntff_profile_via_ctypes("/opt/axon/libaxon_pjrt.so")
        )
    except Exception:
        mod._hook = None


_install_ntff_hook()

from concourse import bass, bacc, mybir, tile  # noqa: E402
from concourse.masks import make_identity  # noqa: E402

try:
    import ml_dtypes

    BF16_NP = ml_dtypes.bfloat16
except ImportError:  # pragma: no cover
    import jax.numpy as jnp

    BF16_NP = jnp.bfloat16


# ---------------------------------------------------------------- config
class Cfg:
    NC = 8          # cores
    N = 100000      # real nodes
    D = 256         # feature dim (= IN_DIM = HID)
    RPC = 12800     # rows per core (padded: 8*12800 = 102400)
    NBLK = 100      # dest blocks of 128 rows per core
    BW = 25600      # source-bucket width (<= 32768 for int16 local indices)
    CHUNK = 10      # blocks per sideband DMA chunk
    SUBMAX = 768    # max indices per dma_gather call (HW ring limit)

    @property
    def NPAD(self):
        return self.NC * self.RPC

    @property
    def NBUCK(self):
        return -(-self.NPAD // self.BW)


CFG = Cfg()


# ---------------------------------------------------------------- builder
def build_nc(cfg: Cfg, caps: tuple):
    """caps[q] = per-(block, bucket-q) edge capacity, multiple of 128."""
    f32 = mybir.dt.float32
    bf16 = mybir.dt.bfloat16
    i16 = mybir.dt.int16
    D, NBLK, CHUNK = cfg.D, cfg.NBLK, cfg.CHUNK
    NBUCK = cfg.NBUCK
    assert len(caps) == NBUCK and all(c % 128 == 0 for c in caps)
    assert NBLK % CHUNK == 0
    TPB = sum(caps) // 128          # edge tiles per block
    IPB = sum(caps) // 16           # int16 idx columns per block
    TOFF = [sum(caps[:q]) // 128 for q in range(NBUCK)]
    IOFF = [sum(caps[:q]) // 16 for q in range(NBUCK)]

    nc = bacc.Bacc(
        "TRN2",
        target_bir_lowering=False,
        debug=False,
        num_devices=cfg.NC,
        num_swdge_queues=min(4, NBUCK),
    )

    xt = nc.dram_tensor("xt", [D, cfg.RPC], bf16, kind="ExternalInput")
    wi = nc.dram_tensor("wi", [D, D], bf16, kind="ExternalInput")  # W_in.T
    w1 = nc.dram_tensor("w1", [D, D], bf16, kind="ExternalInput")  # W_mp1.T
    w2 = nc.dram_tensor("w2", [D, D], bf16, kind="ExternalInput")  # W_mp2.T
    wo = nc.dram_tensor("wo", [128, D // 128], bf16, kind="ExternalInput")
    bin_ = nc.dram_tensor("bin", [1, D], bf16, kind="ExternalInput")
    bout = nc.dram_tensor("bout", [1, 1], bf16, kind="ExternalInput")
    eidx = nc.dram_tensor("eidx", [128, NBLK * IPB], i16, kind="ExternalInput")
    edloc = nc.dram_tensor("edloc", [128, NBLK * TPB], bf16, kind="ExternalInput")
    evals = nc.dram_tensor("evals", [128, NBLK * TPB], bf16, kind="ExternalInput")
    out = nc.dram_tensor("out", [NBLK, 128], f32, kind="ExternalOutput")

    h_shard = nc.dram_tensor("h_shard", [cfg.RPC, D], bf16, kind="Internal")
    h_full = nc.dram_tensor(
        "h_full", [cfg.NPAD, D], bf16, kind="Internal", addr_space="Shared"
    )
    ah_shard = nc.dram_tensor("ah_shard", [cfg.RPC, D], bf16, kind="Internal")
    ah_full = nc.dram_tensor(
        "ah_full", [cfg.NPAD, D], bf16, kind="Internal", addr_space="Shared"
    )
    RG = [list(range(cfg.NC))]
    KD = D // 128  # k-chunks of the feature dim (=2)

    with TileKernel(nc) as tk:
        const = tk.pool("const", bufs=1)
        side = tk.pool("side", bufs=2)
        gpool = tk.pool("gpool", bufs=3)
        mpool = tk.pool("mpool", bufs=6)
        spool = tk.pool("spool", bufs=2)
        psA = tk.pool("psA", bufs=2, space="PSUM")
        psT = tk.pool("psT", bufs=2, space="PSUM")
        psH = tk.pool("psH", bufs=2, space="PSUM")
        psL = tk.pool("psL", bufs=1, space="PSUM")

        # ---------------- constants / weights in SBUF
        wi_sb = [const.tile([128, D], bf16, tag=f"wi{k}", name=f"wi_sb{k}")
                 for k in range(KD)]
        w1_sb = [const.tile([128, D], bf16, tag=f"w1{k}", name=f"w1_sb{k}")
                 for k in range(KD)]
        w2_sb = [const.tile([128, D], bf16, tag=f"w2{k}", name=f"w2_sb{k}")
                 for k in range(KD)]
        for k in range(KD):
            nc.sync.dma_start(out=wi_sb[k][:], in_=wi[k * 128 : (k + 1) * 128, :])
            nc.sync.dma_start(out=w1_sb[k][:], in_=w1[k * 128 : (k + 1) * 128, :])
            nc.sync.dma_start(out=w2_sb[k][:], in_=w2[k * 128 : (k + 1) * 128, :])
        wo_sb = const.tile([128, KD], bf16, tag="wo")
        nc.sync.dma_start(out=wo_sb[:], in_=wo[:])
        bin_sb = const.tile([1, D], bf16, tag="bin")
        nc.sync.dma_start(out=bin_sb[:], in_=bin_[:])
        bout_sb = const.tile([1, 1], bf16, tag="bout")
        nc.sync.dma_start(out=bout_sb[:], in_=bout[:])
        ones1 = const.tile([1, 128], bf16, tag="ones1")
        nc.gpsimd.memset(ones1[:], 1.0)
        iota_i = const.tile([128, 128], mybir.dt.int32, tag="iotai")
        nc.gpsimd.iota(iota_i[:], pattern=[[1, 128]], base=0, channel_multiplier=0)
        NTMAX = cfg.SUBMAX // 128
        iota_rep_i = const.tile([128, NTMAX * 128], mybir.dt.int32, tag="iotari")
        nc.gpsimd.iota(iota_rep_i[:], pattern=[[0, NTMAX], [1, 128]], base=0,
                       channel_multiplier=0)
        iota_rep = const.tile([128, NTMAX * 128], bf16, tag="iotarep")
        nc.vector.tensor_copy(out=iota_rep[:], in_=iota_rep_i[:])
        ident = const.tile([128, 128], bf16, tag="ident")
        make_identity(nc, ident[:])
        identf = const.tile([128, 128], f32, tag="identf")
        make_identity(nc, identf[:])

        xt_sb = [const.tile([128, cfg.RPC], bf16, tag=f"xt{k}", name=f"xt_sb{k}")
                 for k in range(KD)]
        for k in range(KD):
            nc.sync.dma_start(out=xt_sb[k][:], in_=xt[k * 128 : (k + 1) * 128, :])

        aht_res = const.tile([128, NBLK * D], bf16, tag="aht")
        stage = const.tile([128, NBLK], f32, tag="stage")

        relu = mybir.ActivationFunctionType.Relu

        # ---------------- phase A: H = relu(X @ W_in.T + b_in) on own shard
        for b in range(NBLK):
            ph = psA.tile([128, D], f32, tag="acc", name=f"ph_{b}")
            for k in range(KD):
                nc.tensor.matmul(
                    ph[:],
                    lhsT=xt_sb[k][:, b * 128 : (b + 1) * 128],
                    rhs=wi_sb[k][:],
                    start=(k == 0),
                    stop=False,
                )
            nc.tensor.matmul(
                ph[:], lhsT=ones1[:], rhs=bin_sb[:], start=False, stop=True
            )
            h_sb = spool.tile([128, D], bf16, tag="hsb", name=f"h_sb_{b}")
            nc.scalar.activation(out=h_sb[:], in_=ph[:], func=relu)
            nc.sync.dma_start(
                out=h_shard[b * 128 : (b + 1) * 128, :], in_=h_sb[:]
            )

        nc.gpsimd.collective_compute(
            "AllGather",
            mybir.AluOpType.bypass,
            replica_groups=RG,
            ins=[h_shard[:]],
            outs=[h_full[:]],
        )

        # ---------------- SpMM via bucketed dma_gather + one-hot matmul
        def spmm_blocks(src_full, epilogue, phase):
            qrr = [0]
            for c0 in range(0, NBLK, CHUNK):
                i_sb = side.tile([128, CHUNK * IPB], i16, tag="eidx",
                                 name=f"i_sb_{phase}_{c0}")
                d_sb = side.tile([128, CHUNK * TPB], bf16, tag="edloc",
                                 name=f"d_sb_{phase}_{c0}")
                v_sb = side.tile([128, CHUNK * TPB], bf16, tag="evals",
                                 name=f"v_sb_{phase}_{c0}")
                nc.sync.dma_start(
                    out=i_sb[:], in_=eidx[:, c0 * IPB : (c0 + CHUNK) * IPB]
                )
                nc.sync.dma_start(
                    out=d_sb[:], in_=edloc[:, c0 * TPB : (c0 + CHUNK) * TPB]
                )
                nc.sync.dma_start(
                    out=v_sb[:], in_=evals[:, c0 * TPB : (c0 + CHUNK) * TPB]
                )
                for rb in range(CHUNK):
                    b = c0 + rb
                    pacc = psA.tile([128, D], f32, tag="acc",
                                    name=f"pacc_{phase}_{b}")
                    Gs = []
                    for q in range(NBUCK):
                        Tq = caps[q] // 128
                        base = q * cfg.BW
                        bsz = min(cfg.BW, cfg.NPAD - base)
                        G = gpool.tile([128, Tq * D], bf16, tag=f"G{q}",
                                       name=f"G_{phase}_{b}_{q}")
                        io = rb * IPB + IOFF[q]
                        done = 0
                        while done < caps[q]:
                            S = min(cfg.SUBMAX, caps[q] - done)
                            a = done // 128
                            nc.gpsimd.dma_gather(
                                G[:, a * D : (a + S // 128) * D].rearrange(
                                    "p (t d) -> p t d", d=D
                                ),
                                src_full[base : base + bsz, :],
                                i_sb[:, io + a * 8 : io + a * 8 + S // 16],
                                S,
                                S,
                                D,
                                queue_num=qrr[0] % nc.num_swdge_queues,
                            )
                            qrr[0] += 1
                            done += S
                        Gs.append(G)
                    for q in range(NBUCK):
                        Tq = caps[q] // 128
                        a = 0
                        while a < Tq:
                            nt = min(cfg.SUBMAX // 128, Tq - a)
                            j0 = rb * TPB + TOFF[q] + a
                            eq = mpool.tile([128, nt * 128], bf16, tag="EQ",
                                            name=f"eq_{phase}_{b}_{q}_{a}")
                            nc.vector.tensor_tensor(
                                out=eq[:],
                                in0=iota_rep[:, : nt * 128],
                                in1=d_sb[:, j0 : j0 + nt].to_broadcast(
                                    [128, nt, 128]
                                ),
                                op=mybir.AluOpType.is_equal,
                            )
                            m = mpool.tile([128, nt * 128], bf16, tag="M",
                                           name=f"m_{phase}_{b}_{q}_{a}")
                            nc.vector.tensor_tensor(
                                out=m[:],
                                in0=eq[:],
                                in1=v_sb[:, j0 : j0 + nt].to_broadcast(
                                    [128, nt, 128]
                                ),
                                op=mybir.AluOpType.mult,
                            )
                            for tq in range(nt):
                                t = TOFF[q] + a + tq
                                nc.tensor.matmul(
                                    pacc[:],
                                    lhsT=m[:, tq * 128 : (tq + 1) * 128],
                                    rhs=Gs[q][:, (a + tq) * D : (a + tq + 1) * D],
                                    start=(t == 0),
                                    stop=(t == TPB - 1),
                                )
                            a += nt
                    epilogue(b, pacc)

        # ---------------- phase B: AH = A @ H  (store shard + transposed copy)
        def epi_ah(b, pacc):
            ah_sb = spool.tile([128, D], bf16, tag="ahsb", name=f"ah_sb_{b}")
            nc.vector.tensor_copy(out=ah_sb[:], in_=pacc[:])
            nc.sync.dma_start(
                out=ah_shard[b * 128 : (b + 1) * 128, :], in_=ah_sb[:]
            )
            for k in range(KD):
                pt = psT.tile([128, 128], bf16, tag="tp", name=f"ptb_{b}_{k}")
                nc.tensor.transpose(
                    out=pt[:],
                    in_=ah_sb[:, k * 128 : (k + 1) * 128],
                    identity=ident[:],
                )
                nc.vector.tensor_copy(
                    out=aht_res[:, b * D + k * 128 : b * D + (k + 1) * 128],
                    in_=pt[:],
                )

        spmm_blocks(h_full, epi_ah, "B")

        nc.gpsimd.collective_compute(
            "AllGather",
            mybir.AluOpType.bypass,
            replica_groups=RG,
            ins=[ah_shard[:]],
            outs=[ah_full[:]],
        )

        # ---------------- phase C: A2H + dense head, fused per block
        def epi_head(b, pacc):
            a2h_sb = spool.tile([128, D], bf16, tag="a2hsb", name=f"a2h_sb_{b}")
            nc.vector.tensor_copy(out=a2h_sb[:], in_=pacc[:])
            a2ht_sb = spool.tile([128, D], bf16, tag="a2ht", name=f"a2ht_sb_{b}")
            for k in range(KD):
                pt = psT.tile([128, 128], bf16, tag="tp", name=f"ptc_{b}_{k}")
                nc.tensor.transpose(
                    out=pt[:],
                    in_=a2h_sb[:, k * 128 : (k + 1) * 128],
                    identity=ident[:],
                )
                nc.vector.tensor_copy(
                    out=a2ht_sb[:, k * 128 : (k + 1) * 128], in_=pt[:]
                )
            # H2.T = relu(W1 @ AH.T + W2 @ A2H.T), by 128-row halves of h2
            h2t_sb = spool.tile([128, D], bf16, tag="h2t", name=f"h2t_sb_{b}")
            for jj in range(KD):
                ph2 = psH.tile([128, 128], f32, tag="h2", name=f"ph2_{b}_{jj}")
                for k in range(KD):
                    nc.tensor.matmul(
                        ph2[:],
                        lhsT=w1_sb[k][:, jj * 128 : (jj + 1) * 128],
                        rhs=aht_res[:, b * D + k * 128 : b * D + (k + 1) * 128],
                        start=(k == 0),
                        stop=False,
                    )
                    nc.tensor.matmul(
                        ph2[:],
                        lhsT=w2_sb[k][:, jj * 128 : (jj + 1) * 128],
                        rhs=a2ht_sb[:, k * 128 : (k + 1) * 128],
                        start=False,
                        stop=(k == KD - 1),
                    )
                nc.scalar.activation(
                    out=h2t_sb[:, jj * 128 : (jj + 1) * 128], in_=ph2[:], func=relu
                )
            # logits[r] = H2[r,:] @ W_out.T + b_out
            plg = psL.tile([128, 1], f32, tag="lg", name=f"plg_{b}")
            for jj in range(KD):
                nc.tensor.matmul(
                    plg[:],
                    lhsT=h2t_sb[:, jj * 128 : (jj + 1) * 128],
                    rhs=wo_sb[:, jj : jj + 1],
                    start=(jj == 0),
                    stop=False,
                )
            nc.tensor.matmul(
                plg[:], lhsT=ones1[:], rhs=bout_sb[:], start=False, stop=True
            )
            nc.vector.tensor_copy(out=stage[:, b : b + 1], in_=plg[:])

        spmm_blocks(ah_full, epi_head, "C")

        # ---------------- output: transpose stage [128, NBLK] -> [NBLK, 128]
        pso = psL.tile([NBLK, 128], f32, tag="outp")
        nc.tensor.transpose(out=pso[:], in_=stage[:], identity=identf[:])
        out_sb = spool.tile([NBLK, 128], f32, tag="outsb")
        nc.vector.tensor_copy(out=out_sb[:], in_=pso[:])
        nc.sync.dma_start(out=out[:], in_=out_sb[:])

    nc.compile()
    return nc


class TileKernel:
    """Small helper wrapping TileContext + pools in one ExitStack."""

    def __init__(self, nc):
        self.nc = nc
        self._ctx = ExitStack()
        self.tc = None

    def __enter__(self):
        self.tc = self._ctx.enter_context(tile.TileContext(self.nc))
        return self

    def pool(self, name, bufs, space="SBUF"):
        return self._ctx.enter_context(
            self.tc.tile_pool(name=name, bufs=bufs, space=space)
        )

    def __exit__(self, *exc):
        return self._ctx.__exit__(*exc)


# ---------------------------------------------------------------- host prep
def edge_caps(rows, cols, cfg: Cfg):
    """Per-bucket capacity = max over (core, dest block) run length, to 128."""
    rows = np.asarray(rows).astype(np.int64)
    cols = np.asarray(cols).astype(np.int64)
    core = rows // cfg.RPC
    blk = (rows % cfg.RPC) // 128
    buck = cols // cfg.BW
    gid = (core * cfg.NBLK + blk) * cfg.NBUCK + buck
    cnt = np.bincount(gid, minlength=cfg.NC * cfg.NBLK * cfg.NBUCK)
    cnt = cnt.reshape(-1, cfg.NBUCK)
    caps = (-(-cnt.max(0) // 128) * 128).astype(np.int64)
    caps = np.maximum(caps, 128)
    return tuple(int(c) for c in caps)


def prep_inputs(X, rows, cols, vals, W_in, b_in, W_mp1, W_mp2, W_out, b_out,
                cfg: Cfg, caps: tuple):
    NC, RPC, NBLK, D = cfg.NC, cfg.RPC, cfg.NBLK, cfg.D
    NBUCK = cfg.NBUCK
    rows = np.asarray(rows).astype(np.int64)
    cols = np.asarray(cols).astype(np.int64)
    vals = np.asarray(vals, dtype=np.float32)
    X = np.asarray(X, dtype=np.float32)
    TPB = sum(caps) // 128
    IPB = sum(caps) // 16
    TOFF = [sum(caps[:q]) // 128 for q in range(NBUCK)]

    core = rows // RPC
    blk = (rows % RPC) // 128
    dloc = (rows % 128).astype(np.float32)
    buck = cols // cfg.BW
    lcol = cols - buck * cfg.BW  # local index within bucket (< BW <= 32768)

    gid = (core * NBLK + blk) * NBUCK + buck
    ngroups = NC * NBLK * NBUCK
    order = np.lexsort((cols, gid))
    counts = np.bincount(gid, minlength=ngroups)
    capv = np.array(caps, dtype=np.int64)
    assert (counts.reshape(-1, NBUCK) <= capv[None, :]).all()
    starts = np.zeros(ngroups, np.int64)
    starts[1:] = np.cumsum(counts)[:-1]
    g_sorted = gid[order]
    pos = np.arange(len(rows), dtype=np.int64) - starts[g_sorted]

    c_s = core[order]
    b_s = blk[order]
    q_s = buck[order]

    # dloc/vals sideband: edge i of its run at (p=i%128, tile=i//128)
    dloc_arr = np.zeros((NC, 128, NBLK * TPB), BF16_NP)
    vals_arr = np.zeros((NC, 128, NBLK * TPB), BF16_NP)
    toff = np.array(TOFF, dtype=np.int64)
    tcol = b_s * TPB + toff[q_s] + pos // 128
    p = pos % 128
    dloc_arr[c_s, p, tcol] = dloc[order].astype(BF16_NP)
    vals_arr[c_s, p, tcol] = vals[order].astype(BF16_NP)

    # int16 gather indices: edge i of its run at (p=i%16 (replicated x8),
    # col = run_icol0 + i//16)
    ioff = np.array([sum(caps[:q]) // 16 for q in range(NBUCK)], dtype=np.int64)
    idx_arr = np.zeros((NC, 16, NBLK * IPB), np.int16)
    SUB = cfg.SUBMAX
    jj = pos % SUB
    icol = b_s * IPB + ioff[q_s] + (pos // SUB) * (SUB // 16) + jj // 16
    ip = jj % 16
    idx_arr[c_s, ip, icol] = lcol[order].astype(np.int16)
    idx_arr = np.tile(idx_arr, (1, 8, 1))  # replicate to 128 partitions

    wi_h = np.ascontiguousarray(np.asarray(W_in, np.float32).T).astype(BF16_NP)
    w1_h = np.ascontiguousarray(np.asarray(W_mp1, np.float32).T).astype(BF16_NP)
    w2_h = np.ascontiguousarray(np.asarray(W_mp2, np.float32).T).astype(BF16_NP)
    wo_h = np.ascontiguousarray(
        np.asarray(W_out, np.float32).reshape(D // 128, 128).T
    ).astype(BF16_NP)
    bin_h = np.asarray(b_in, np.float32).reshape(1, D).astype(BF16_NP)
    bout_h = np.asarray(b_out, np.float32).reshape(1, 1).astype(BF16_NP)

    in_maps = []
    for c in range(NC):
        lo = c * RPC
        hi = min((c + 1) * RPC, cfg.N)
        xs = np.zeros((RPC, D), np.float32)
        if hi > lo:
            xs[: hi - lo] = X[lo:hi]
        xt_h = np.ascontiguousarray(xs.T).astype(BF16_NP)
        in_maps.append(
            {
                "xt": xt_h,
                "wi": wi_h,
                "w1": w1_h,
                "w2": w2_h,
                "wo": wo_h,
                "bin": bin_h,
                "bout": bout_h,
                "eidx": idx_arr[c],
                "edloc": dloc_arr[c],
                "evals": vals_arr[c],
            }
        )
    return in_maps


# ---------------------------------------------------------------- entry
_NC_CACHE = {}


def _get_nc(cfg: Cfg, caps: tuple):
    key = (cfg.NC, cfg.RPC, cfg.NBLK, cfg.BW, cfg.CHUNK, caps)
    if key not in _NC_CACHE:
        _NC_CACHE[key] = build_nc(cfg, caps)
    return _NC_CACHE[key]


def run(inputs, trace=False):
    from concourse.bass_utils import run_bass_kernel_spmd

    cfg = CFG
    caps = edge_caps(inputs["rows"], inputs["cols"], cfg)
    nc = _get_nc(cfg, caps)
    in_maps = prep_inputs(**inputs, cfg=cfg, caps=caps)
    res = run_bass_kernel_spmd(
        nc, in_maps, core_ids=list(range(cfg.NC)), trace=trace
    )
    pieces = [res.results[c]["out"].reshape(-1) for c in range(cfg.NC)]
    full = np.concatenate(pieces)[: cfg.N]
    return full.reshape(cfg.N, 1).astype(np.float32), res


def kernel(**inputs):
    out, _ = run(inputs, trace=False)
    return out


if __name__ == "__main__":
    import reference

    inp = {k: np.asarray(v) for k, v in reference.setup_inputs().items()}
    got = kernel(**inp)
    print("kernel output:", got.shape, got.dtype)


# revision 12
# speedup vs baseline: 1.0766x; 1.0766x over previous
"""Trainium2 Bass kernel for BetaGNN message passing (8 NeuronCores).

Strategy:
  - Node rows sharded 8 ways (12800 padded rows per core, 100 blocks of 128).
  - H = relu(X @ W_in.T + b_in) computed per-shard on device (bf16), AllGather.
  - SpMM (A @ H): edges partitioned by destination row on host, grouped by
    (dest block, source bucket) and padded to fixed capacities. Each run is
    one dma_gather (int16 local indices into a <=32768-row bucket of the
    all-gathered H), then segment-summed via one-hot matmuls on the PE
    (M[e,d] = (iota_d == dloc_e) * val_e; PSUM accumulates over edge tiles).
  - AH AllGather, then SpMM2 fused with the dense head; logits per shard;
    host concatenates and trims padding.
"""

import sys
import types
from contextlib import ExitStack

import numpy as np

# ---------------------------------------------------------------- ntff hook
def _install_ntff_hook():
    """The image's antenv lacks axon_hooks; synthesize it so trace=True works."""
    if "antenv.axon_hooks" in sys.modules:
        return
    try:
        import antenv  # noqa: F401
    except ImportError:
        return
    mod = types.ModuleType("antenv.axon_hooks")
    mod._hook = None

    def set_axon_ntff_profile_hook(h):
        mod._hook = h

    def get_axon_ntff_profile_hook():
        return mod._hook

    mod.set_axon_ntff_profile_hook = set_axon_ntff_profile_hook
    mod.get_axon_ntff_profile_hook = get_axon_ntff_profile_hook
    sys.modules["antenv.axon_hooks"] = mod
    try:
        from trn_agent_boot.trn_boot import _ntff_profile_via_ctypes

        set_axon_ntff_profile_hook(
            _ntff_profile_via_ctypes("/opt/axon/libaxon_pjrt.so")
        )
    except Exception:
        mod._hook = None


_install_ntff_hook()

from concourse import bass, bacc, mybir, tile  # noqa: E402
from concourse.masks import make_identity  # noqa: E402

try:
    import ml_dtypes

    BF16_NP = ml_dtypes.bfloat16
except ImportError:  # pragma: no cover
    import jax.numpy as jnp

    BF16_NP = jnp.bfloat16


# ---------------------------------------------------------------- config
class Cfg:
    NC = 8          # cores
    N = 100000      # real nodes
    D = 256         # feature dim (= IN_DIM = HID)
    RPC = 12800     # rows per core (padded: 8*12800 = 102400)
    NBLK = 100      # dest blocks of 128 rows per core
    BW = 25600      # source-bucket width (<= 32768 for int16 local indices)
    CHUNK = 10      # blocks per sideband DMA chunk
    SUBMAX = 768    # max indices per dma_gather call (HW ring limit)

    @property
    def NPAD(self):
        return self.NC * self.RPC

    @property
    def NBUCK(self):
        return -(-self.NPAD // self.BW)


CFG = Cfg()


# ---------------------------------------------------------------- builder
def build_nc(cfg: Cfg, caps: tuple):
    """caps[q] = per-(block, bucket-q) edge capacity, multiple of 128."""
    f32 = mybir.dt.float32
    bf16 = mybir.dt.bfloat16
    i16 = mybir.dt.int16
    D, NBLK, CHUNK = cfg.D, cfg.NBLK, cfg.CHUNK
    NBUCK = cfg.NBUCK
    assert len(caps) == NBUCK and all(c % 128 == 0 for c in caps)
    assert NBLK % CHUNK == 0
    TPB = sum(caps) // 128          # edge tiles per block
    IPB = sum(caps) // 16           # int16 idx columns per block
    TOFF = [sum(caps[:q]) // 128 for q in range(NBUCK)]
    IOFF = [sum(caps[:q]) // 16 for q in range(NBUCK)]

    nc = bacc.Bacc(
        "TRN2",
        target_bir_lowering=False,
        debug=False,
        num_devices=cfg.NC,
        num_swdge_queues=min(4, NBUCK),
    )

    xt = nc.dram_tensor("xt", [D, cfg.RPC], bf16, kind="ExternalInput")
    wi = nc.dram_tensor("wi", [D, D], bf16, kind="ExternalInput")  # W_in.T
    w1 = nc.dram_tensor("w1", [D, D], bf16, kind="ExternalInput")  # W_mp1.T
    w2 = nc.dram_tensor("w2", [D, D], bf16, kind="ExternalInput")  # W_mp2.T
    wo = nc.dram_tensor("wo", [128, D // 128], bf16, kind="ExternalInput")
    bin_ = nc.dram_tensor("bin", [1, D], bf16, kind="ExternalInput")
    bout = nc.dram_tensor("bout", [1, 1], bf16, kind="ExternalInput")
    eidx = nc.dram_tensor("eidx", [128, NBLK * IPB], i16, kind="ExternalInput")
    edloc = nc.dram_tensor("edloc", [128, NBLK * TPB], bf16, kind="ExternalInput")
    evals = nc.dram_tensor("evals", [128, NBLK * TPB], bf16, kind="ExternalInput")
    out = nc.dram_tensor("out", [NBLK, 128], f32, kind="ExternalOutput")

    h_shard = nc.dram_tensor("h_shard", [cfg.RPC, D], bf16, kind="Internal")
    h_full = nc.dram_tensor(
        "h_full", [cfg.NPAD, D], bf16, kind="Internal", addr_space="Shared"
    )
    ah_shard = nc.dram_tensor("ah_shard", [cfg.RPC, D], bf16, kind="Internal")
    ah_full = nc.dram_tensor(
        "ah_full", [cfg.NPAD, D], bf16, kind="Internal", addr_space="Shared"
    )
    RG = [list(range(cfg.NC))]
    KD = D // 128  # k-chunks of the feature dim (=2)

    with TileKernel(nc) as tk:
        const = tk.pool("const", bufs=1)
        side = tk.pool("side", bufs=2)
        gpool = tk.pool("gpool", bufs=3)
        mpool = tk.pool("mpool", bufs=6)
        spool = tk.pool("spool", bufs=2)
        psA = tk.pool("psA", bufs=2, space="PSUM")
        psT = tk.pool("psT", bufs=2, space="PSUM")
        psH = tk.pool("psH", bufs=2, space="PSUM")
        psL = tk.pool("psL", bufs=1, space="PSUM")

        # ---------------- constants / weights in SBUF
        wi_sb = [const.tile([128, D], bf16, tag=f"wi{k}", name=f"wi_sb{k}")
                 for k in range(KD)]
        w1_sb = [const.tile([128, D], bf16, tag=f"w1{k}", name=f"w1_sb{k}")
                 for k in range(KD)]
        w2_sb = [const.tile([128, D], bf16, tag=f"w2{k}", name=f"w2_sb{k}")
                 for k in range(KD)]
        for k in range(KD):
            nc.sync.dma_start(out=wi_sb[k][:], in_=wi[k * 128 : (k + 1) * 128, :])
            nc.sync.dma_start(out=w1_sb[k][:], in_=w1[k * 128 : (k + 1) * 128, :])
            nc.sync.dma_start(out=w2_sb[k][:], in_=w2[k * 128 : (k + 1) * 128, :])
        wo_sb = const.tile([128, KD], bf16, tag="wo")
        nc.sync.dma_start(out=wo_sb[:], in_=wo[:])
        bin_sb = const.tile([1, D], bf16, tag="bin")
        nc.sync.dma_start(out=bin_sb[:], in_=bin_[:])
        bout_sb = const.tile([1, 1], bf16, tag="bout")
        nc.sync.dma_start(out=bout_sb[:], in_=bout[:])
        ones1 = const.tile([1, 128], bf16, tag="ones1")
        nc.gpsimd.memset(ones1[:], 1.0)
        iota_i = const.tile([128, 128], mybir.dt.int32, tag="iotai")
        nc.gpsimd.iota(iota_i[:], pattern=[[1, 128]], base=0, channel_multiplier=0)
        NTMAX = cfg.SUBMAX // 128
        iota_rep_i = const.tile([128, NTMAX * 128], mybir.dt.int32, tag="iotari")
        nc.gpsimd.iota(iota_rep_i[:], pattern=[[0, NTMAX], [1, 128]], base=0,
                       channel_multiplier=0)
        iota_rep = const.tile([128, NTMAX * 128], bf16, tag="iotarep")
        nc.vector.tensor_copy(out=iota_rep[:], in_=iota_rep_i[:])
        ident = const.tile([128, 128], bf16, tag="ident")
        make_identity(nc, ident[:])
        identf = const.tile([128, 128], f32, tag="identf")
        make_identity(nc, identf[:])

        xt_sb = [const.tile([128, cfg.RPC], bf16, tag=f"xt{k}", name=f"xt_sb{k}")
                 for k in range(KD)]
        for k in range(KD):
            nc.sync.dma_start(out=xt_sb[k][:], in_=xt[k * 128 : (k + 1) * 128, :])

        aht_res = const.tile([128, NBLK * D], bf16, tag="aht")
        stage = const.tile([128, NBLK], f32, tag="stage")

        relu = mybir.ActivationFunctionType.Relu

        # ---------------- phase A: H = relu(X @ W_in.T + b_in) on own shard
        for b in range(NBLK):
            ph = psA.tile([128, D], f32, tag="acc", name=f"ph_{b}")
            for k in range(KD):
                nc.tensor.matmul(
                    ph[:],
                    lhsT=xt_sb[k][:, b * 128 : (b + 1) * 128],
                    rhs=wi_sb[k][:],
                    start=(k == 0),
                    stop=False,
                )
            nc.tensor.matmul(
                ph[:], lhsT=ones1[:], rhs=bin_sb[:], start=False, stop=True
            )
            h_sb = spool.tile([128, D], bf16, tag="hsb", name=f"h_sb_{b}")
            nc.scalar.activation(out=h_sb[:], in_=ph[:], func=relu)
            nc.sync.dma_start(
                out=h_shard[b * 128 : (b + 1) * 128, :], in_=h_sb[:]
            )

        nc.gpsimd.collective_compute(
            "AllGather",
            mybir.AluOpType.bypass,
            replica_groups=RG,
            ins=[h_shard[:]],
            outs=[h_full[:]],
        )

        # ---------------- SpMM via bucketed dma_gather + one-hot matmul
        def spmm_blocks(src_full, epilogue, phase):
            qrr = [0]
            for c0 in range(0, NBLK, CHUNK):
                i_sb = side.tile([128, CHUNK * IPB], i16, tag="eidx",
                                 name=f"i_sb_{phase}_{c0}")
                d_sb = side.tile([128, CHUNK * TPB], bf16, tag="edloc",
                                 name=f"d_sb_{phase}_{c0}")
                v_sb = side.tile([128, CHUNK * TPB], bf16, tag="evals",
                                 name=f"v_sb_{phase}_{c0}")
                nc.sync.dma_start(
                    out=i_sb[:], in_=eidx[:, c0 * IPB : (c0 + CHUNK) * IPB]
                )
                nc.sync.dma_start(
                    out=d_sb[:], in_=edloc[:, c0 * TPB : (c0 + CHUNK) * TPB]
                )
                nc.sync.dma_start(
                    out=v_sb[:], in_=evals[:, c0 * TPB : (c0 + CHUNK) * TPB]
                )
                for rb in range(CHUNK):
                    b = c0 + rb
                    pacc = psA.tile([128, D], f32, tag="acc",
                                    name=f"pacc_{phase}_{b}")
                    Gs = []
                    for q in range(NBUCK):
                        Tq = caps[q] // 128
                        base = q * cfg.BW
                        bsz = min(cfg.BW, cfg.NPAD - base)
                        G = gpool.tile([128, Tq * D], bf16, tag=f"G{q}",
                                       name=f"G_{phase}_{b}_{q}")
                        io = rb * IPB + IOFF[q]
                        done = 0
                        while done < caps[q]:
                            S = min(cfg.SUBMAX, caps[q] - done)
                            a = done // 128
                            nc.gpsimd.dma_gather(
                                G[:, a * D : (a + S // 128) * D].rearrange(
                                    "p (t d) -> p t d", d=D
                                ),
                                src_full[base : base + bsz, :],
                                i_sb[:, io + a * 8 : io + a * 8 + S // 16],
                                S,
                                S,
                                D,
                                queue_num=qrr[0] % nc.num_swdge_queues,
                            )
                            qrr[0] += 1
                            done += S
                        Gs.append(G)
                    for q in range(NBUCK):
                        Tq = caps[q] // 128
                        a = 0
                        while a < Tq:
                            nt = min(cfg.SUBMAX // 128, Tq - a)
                            j0 = rb * TPB + TOFF[q] + a
                            eq = mpool.tile([128, nt * 128], bf16, tag="EQ",
                                            name=f"eq_{phase}_{b}_{q}_{a}")
                            nc.vector.tensor_tensor(
                                out=eq[:],
                                in0=iota_rep[:, : nt * 128],
                                in1=d_sb[:, j0 : j0 + nt].to_broadcast(
                                    [128, nt, 128]
                                ),
                                op=mybir.AluOpType.is_equal,
                            )
                            m = mpool.tile([128, nt * 128], bf16, tag="M",
                                           name=f"m_{phase}_{b}_{q}_{a}")
                            nc.vector.tensor_tensor(
                                out=m[:],
                                in0=eq[:],
                                in1=v_sb[:, j0 : j0 + nt].to_broadcast(
                                    [128, nt, 128]
                                ),
                                op=mybir.AluOpType.mult,
                            )
                            for tq in range(nt):
                                t = TOFF[q] + a + tq
                                nc.tensor.matmul(
                                    pacc[:],
                                    lhsT=m[:, tq * 128 : (tq + 1) * 128],
                                    rhs=Gs[q][:, (a + tq) * D : (a + tq + 1) * D],
                                    start=(t == 0),
                                    stop=(t == TPB - 1),
                                )
                            a += nt
                    epilogue(b, pacc)

        # ---------------- phase B: AH = A @ H  (store shard + transposed copy)
        def epi_ah(b, pacc):
            ah_sb = spool.tile([128, D], bf16, tag="ahsb", name=f"ah_sb_{b}")
            nc.vector.tensor_copy(out=ah_sb[:], in_=pacc[:])
            nc.sync.dma_start(
                out=ah_shard[b * 128 : (b + 1) * 128, :], in_=ah_sb[:]
            )
            for k in range(KD):
                pt = psT.tile([128, 128], bf16, tag="tp", name=f"ptb_{b}_{k}")
                nc.tensor.transpose(
                    out=pt[:],
                    in_=ah_sb[:, k * 128 : (k + 1) * 128],
                    identity=ident[:],
                )
                nc.vector.tensor_copy(
                    out=aht_res[:, b * D + k * 128 : b * D + (k + 1) * 128],
                    in_=pt[:],
                )

        spmm_blocks(h_full, epi_ah, "B")

        nc.gpsimd.collective_compute(
            "AllGather",
            mybir.AluOpType.bypass,
            replica_groups=RG,
            ins=[ah_shard[:]],
            outs=[ah_full[:]],
        )

        # ---------------- phase C: A2H + dense head, fused per block
        def epi_head(b, pacc):
            a2h_sb = spool.tile([128, D], bf16, tag="a2hsb", name=f"a2h_sb_{b}")
            nc.vector.tensor_copy(out=a2h_sb[:], in_=pacc[:])
            a2ht_sb = spool.tile([128, D], bf16, tag="a2ht", name=f"a2ht_sb_{b}")
            for k in range(KD):
                pt = psT.tile([128, 128], bf16, tag="tp", name=f"ptc_{b}_{k}")
                nc.tensor.transpose(
                    out=pt[:],
                    in_=a2h_sb[:, k * 128 : (k + 1) * 128],
                    identity=ident[:],
                )
                nc.vector.tensor_copy(
                    out=a2ht_sb[:, k * 128 : (k + 1) * 128], in_=pt[:]
                )
            # H2.T = relu(W1 @ AH.T + W2 @ A2H.T), by 128-row halves of h2
            h2t_sb = spool.tile([128, D], bf16, tag="h2t", name=f"h2t_sb_{b}")
            for jj in range(KD):
                ph2 = psH.tile([128, 128], f32, tag="h2", name=f"ph2_{b}_{jj}")
                for k in range(KD):
                    nc.tensor.matmul(
                        ph2[:],
                        lhsT=w1_sb[k][:, jj * 128 : (jj + 1) * 128],
                        rhs=aht_res[:, b * D + k * 128 : b * D + (k + 1) * 128],
                        start=(k == 0),
                        stop=False,
                    )
                    nc.tensor.matmul(
                        ph2[:],
                        lhsT=w2_sb[k][:, jj * 128 : (jj + 1) * 128],
                        rhs=a2ht_sb[:, k * 128 : (k + 1) * 128],
                        start=False,
                        stop=(k == KD - 1),
                    )
                nc.scalar.activation(
                    out=h2t_sb[:, jj * 128 : (jj + 1) * 128], in_=ph2[:], func=relu
                )
            # logits[r] = H2[r,:] @ W_out.T + b_out
            plg = psL.tile([128, 1], f32, tag="lg", name=f"plg_{b}")
            for jj in range(KD):
                nc.tensor.matmul(
                    plg[:],
                    lhsT=h2t_sb[:, jj * 128 : (jj + 1) * 128],
                    rhs=wo_sb[:, jj : jj + 1],
                    start=(jj == 0),
                    stop=False,
                )
            nc.tensor.matmul(
                plg[:], lhsT=ones1[:], rhs=bout_sb[:], start=False, stop=True
            )
            nc.vector.tensor_copy(out=stage[:, b : b + 1], in_=plg[:])

        spmm_blocks(ah_full, epi_head, "C")

        # ---------------- output: transpose stage [128, NBLK] -> [NBLK, 128]
        pso = psL.tile([NBLK, 128], f32, tag="outp")
        nc.tensor.transpose(out=pso[:], in_=stage[:], identity=identf[:])
        out_sb = spool.tile([NBLK, 128], f32, tag="outsb")
        nc.vector.tensor_copy(out=out_sb[:], in_=pso[:])
        nc.sync.dma_start(out=out[:], in_=out_sb[:])

    nc.compile()
    return nc


class TileKernel:
    """Small helper wrapping TileContext + pools in one ExitStack."""

    def __init__(self, nc):
        self.nc = nc
        self._ctx = ExitStack()
        self.tc = None

    def __enter__(self):
        self.tc = self._ctx.enter_context(tile.TileContext(self.nc))
        return self

    def pool(self, name, bufs, space="SBUF"):
        return self._ctx.enter_context(
            self.tc.tile_pool(name=name, bufs=bufs, space=space)
        )

    def __exit__(self, *exc):
        return self._ctx.__exit__(*exc)


# ---------------------------------------------------------------- host prep
def edge_caps(rows, cols, cfg: Cfg):
    """Per-bucket capacity = max over (core, dest block) run length, to 128."""
    rows = np.asarray(rows).astype(np.int64)
    cols = np.asarray(cols).astype(np.int64)
    core = rows // cfg.RPC
    blk = (rows % cfg.RPC) // 128
    buck = cols // cfg.BW
    gid = (core * cfg.NBLK + blk) * cfg.NBUCK + buck
    cnt = np.bincount(gid, minlength=cfg.NC * cfg.NBLK * cfg.NBUCK)
    cnt = cnt.reshape(-1, cfg.NBUCK)
    caps = (-(-cnt.max(0) // 128) * 128).astype(np.int64)
    caps = np.maximum(caps, 128)
    return tuple(int(c) for c in caps)


def prep_inputs(X, rows, cols, vals, W_in, b_in, W_mp1, W_mp2, W_out, b_out,
                cfg: Cfg, caps: tuple):
    NC, RPC, NBLK, D = cfg.NC, cfg.RPC, cfg.NBLK, cfg.D
    NBUCK = cfg.NBUCK
    rows = np.asarray(rows).astype(np.int64)
    cols = np.asarray(cols).astype(np.int64)
    vals = np.asarray(vals, dtype=np.float32)
    X = np.asarray(X, dtype=np.float32)
    TPB = sum(caps) // 128
    IPB = sum(caps) // 16
    TOFF = [sum(caps[:q]) // 128 for q in range(NBUCK)]

    core = rows // RPC
    blk = (rows % RPC) // 128
    dloc = (rows % 128).astype(np.float32)
    buck = cols // cfg.BW
    lcol = cols - buck * cfg.BW  # local index within bucket (< BW <= 32768)

    gid = (core * NBLK + blk) * NBUCK + buck
    ngroups = NC * NBLK * NBUCK
    order = np.lexsort((cols, gid))
    counts = np.bincount(gid, minlength=ngroups)
    capv = np.array(caps, dtype=np.int64)
    assert (counts.reshape(-1, NBUCK) <= capv[None, :]).all()
    starts = np.zeros(ngroups, np.int64)
    starts[1:] = np.cumsum(counts)[:-1]
    g_sorted = gid[order]
    pos = np.arange(len(rows), dtype=np.int64) - starts[g_sorted]

    c_s = core[order]
    b_s = blk[order]
    q_s = buck[order]

    # dloc/vals sideband: edge i of its run at (p=i%128, tile=i//128)
    dloc_arr = np.zeros((NC, 128, NBLK * TPB), BF16_NP)
    vals_arr = np.zeros((NC, 128, NBLK * TPB), BF16_NP)
    toff = np.array(TOFF, dtype=np.int64)
    tcol = b_s * TPB + toff[q_s] + pos // 128
    p = pos % 128
    dloc_arr[c_s, p, tcol] = dloc[order].astype(BF16_NP)
    vals_arr[c_s, p, tcol] = vals[order].astype(BF16_NP)

    # int16 gather indices: edge i of its run at (p=i%16 (replicated x8),
    # col = run_icol0 + i//16)
    ioff = np.array([sum(caps[:q]) // 16 for q in range(NBUCK)], dtype=np.int64)
    idx_arr = np.zeros((NC, 16, NBLK * IPB), np.int16)
    SUB = cfg.SUBMAX
    jj = pos % SUB
    icol = b_s * IPB + ioff[q_s] + (pos // SUB) * (SUB // 16) + jj // 16
    ip = jj % 16
    idx_arr[c_s, ip, icol] = lcol[order].astype(np.int16)
    idx_arr = np.tile(idx_arr, (1, 8, 1))  # replicate to 128 partitions

    wi_h = np.ascontiguousarray(np.asarray(W_in, np.float32).T).astype(BF16_NP)
    w1_h = np.ascontiguousarray(np.asarray(W_mp1, np.float32).T).astype(BF16_NP)
    w2_h = np.ascontiguousarray(np.asarray(W_mp2, np.float32).T).astype(BF16_NP)
    wo_h = np.ascontiguousarray(
        np.asarray(W_out, np.float32).reshape(D // 128, 128).T
    ).astype(BF16_NP)
    bin_h = np.asarray(b_in, np.float32).reshape(1, D).astype(BF16_NP)
    bout_h = np.asarray(b_out, np.float32).reshape(1, 1).astype(BF16_NP)

    in_maps = []
    for c in range(NC):
        lo = c * RPC
        hi = min((c + 1) * RPC, cfg.N)
        xs = np.zeros((RPC, D), np.float32)
        if hi > lo:
            xs[: hi - lo] = X[lo:hi]
        xt_h = np.ascontiguousarray(xs.T).astype(BF16_NP)
        in_maps.append(
            {
                "xt": xt_h,
                "wi": wi_h,
                "w1": w1_h,
                "w2": w2_h,
                "wo": wo_h,
                "bin": bin_h,
                "bout": bout_h,
                "eidx": idx_arr[c],
                "edloc": dloc_arr[c],
                "evals": vals_arr[c],
            }
        )
    return in_maps


# ---------------------------------------------------------------- entry
_NC_CACHE = {}


def _get_nc(cfg: Cfg, caps: tuple):
    key = (cfg.NC, cfg.RPC, cfg.NBLK, cfg.BW, cfg.CHUNK, caps)
    if key not in _NC_CACHE:
        _NC_CACHE[key] = build_nc(cfg, caps)
    return _NC_CACHE[key]


def run(inputs, trace=False):
    from concourse.bass_utils import run_bass_kernel_spmd

    cfg = CFG
    caps = edge_caps(inputs["rows"], inputs["cols"], cfg)
    nc = _get_nc(cfg, caps)
    in_maps = prep_inputs(**inputs, cfg=cfg, caps=caps)
    res = run_bass_kernel_spmd(
        nc, in_maps, core_ids=list(range(cfg.NC)), trace=trace
    )
    pieces = [res.results[c]["out"].reshape(-1) for c in range(cfg.NC)]
    full = np.concatenate(pieces)[: cfg.N]
    return full.reshape(cfg.N, 1).astype(np.float32), res


def kernel(**inputs):
    out, _ = run(inputs, trace=False)
    return out


if __name__ == "__main__":
    import reference

    inp = {k: np.asarray(v) for k, v in reference.setup_inputs().items()}
    got = kernel(**inp)
    print("kernel output:", got.shape, got.dtype)


# revision 13
# speedup vs baseline: 1.0811x; 1.0042x over previous
"""Trainium2 Bass kernel for BetaGNN message passing (8 NeuronCores).

Strategy:
  - Node rows sharded 8 ways (12800 padded rows per core, 100 blocks of 128).
  - H = relu(X @ W_in.T + b_in) computed per-shard on device (bf16), AllGather.
  - SpMM (A @ H): edges partitioned by destination row on host, grouped by
    (dest block, source bucket) and padded to fixed capacities. Each run is
    one dma_gather (int16 local indices into a <=32768-row bucket of the
    all-gathered H), then segment-summed via one-hot matmuls on the PE
    (M[e,d] = (iota_d == dloc_e) * val_e; PSUM accumulates over edge tiles).
  - AH AllGather, then SpMM2 fused with the dense head; logits per shard;
    host concatenates and trims padding.
"""

import sys
import types
from contextlib import ExitStack

import numpy as np

# ---------------------------------------------------------------- ntff hook
def _install_ntff_hook():
    """The image's antenv lacks axon_hooks; synthesize it so trace=True works."""
    if "antenv.axon_hooks" in sys.modules:
        return
    try:
        import antenv  # noqa: F401
    except ImportError:
        return
    mod = types.ModuleType("antenv.axon_hooks")
    mod._hook = None

    def set_axon_ntff_profile_hook(h):
        mod._hook = h

    def get_axon_ntff_profile_hook():
        return mod._hook

    mod.set_axon_ntff_profile_hook = set_axon_ntff_profile_hook
    mod.get_axon_ntff_profile_hook = get_axon_ntff_profile_hook
    sys.modules["antenv.axon_hooks"] = mod
    try:
        from trn_agent_boot.trn_boot import _ntff_profile_via_ctypes

        set_axon_ntff_profile_hook(
            _ntff_profile_via_ctypes("/opt/axon/libaxon_pjrt.so")
        )
    except Exception:
        mod._hook = None


_install_ntff_hook()

from concourse import bass, bacc, mybir, tile  # noqa: E402
from concourse.masks import make_identity  # noqa: E402

try:
    import ml_dtypes

    BF16_NP = ml_dtypes.bfloat16
except ImportError:  # pragma: no cover
    import jax.numpy as jnp

    BF16_NP = jnp.bfloat16


# ---------------------------------------------------------------- config
class Cfg:
    NC = 8          # cores
    N = 100000      # real nodes
    D = 256         # feature dim (= IN_DIM = HID)
    RPC = 12800     # rows per core (padded: 8*12800 = 102400)
    NBLK = 100      # dest blocks of 128 rows per core
    BW = 25600      # source-bucket width (<= 32768 for int16 local indices)
    CHUNK = 10      # blocks per sideband DMA chunk
    SUBMAX = 768    # max indices per dma_gather call (HW ring limit)

    @property
    def NPAD(self):
        return self.NC * self.RPC

    @property
    def NBUCK(self):
        return -(-self.NPAD // self.BW)


CFG = Cfg()


# ---------------------------------------------------------------- builder
def build_nc(cfg: Cfg, caps: tuple):
    """caps[q] = per-(block, bucket-q) edge capacity, multiple of 128."""
    f32 = mybir.dt.float32
    bf16 = mybir.dt.bfloat16
    i16 = mybir.dt.int16
    D, NBLK, CHUNK = cfg.D, cfg.NBLK, cfg.CHUNK
    NBUCK = cfg.NBUCK
    assert len(caps) == NBUCK and all(c % 128 == 0 for c in caps)
    assert cfg.BW == cfg.NC * (cfg.RPC // NBUCK), "bucket width must match chunk layout"
    assert NBLK % CHUNK == 0
    TPB = sum(caps) // 128          # edge tiles per block
    IPB = sum(caps) // 16           # int16 idx columns per block
    TOFF = [sum(caps[:q]) // 128 for q in range(NBUCK)]
    IOFF = [sum(caps[:q]) // 16 for q in range(NBUCK)]

    nc = bacc.Bacc(
        "TRN2",
        target_bir_lowering=False,
        debug=False,
        num_devices=cfg.NC,
        num_swdge_queues=min(4, NBUCK),
    )

    xt = nc.dram_tensor("xt", [D, cfg.RPC], bf16, kind="ExternalInput")
    wi = nc.dram_tensor("wi", [D, D], bf16, kind="ExternalInput")  # W_in.T
    w1 = nc.dram_tensor("w1", [D, D], bf16, kind="ExternalInput")  # W_mp1.T
    w2 = nc.dram_tensor("w2", [D, D], bf16, kind="ExternalInput")  # W_mp2.T
    wo = nc.dram_tensor("wo", [128, D // 128], bf16, kind="ExternalInput")
    bin_ = nc.dram_tensor("bin", [1, D], bf16, kind="ExternalInput")
    bout = nc.dram_tensor("bout", [1, 1], bf16, kind="ExternalInput")
    eidx = nc.dram_tensor("eidx", [128, NBLK * IPB], i16, kind="ExternalInput")
    edloc = nc.dram_tensor("edloc", [128, NBLK * TPB], bf16, kind="ExternalInput")
    evals = nc.dram_tensor("evals", [128, NBLK * TPB], bf16, kind="ExternalInput")
    out = nc.dram_tensor("out", [NBLK, 128], f32, kind="ExternalOutput")

    NCH = cfg.NBUCK  # collective chunks == gather buckets
    CROWS = cfg.RPC // NCH          # shard rows per chunk
    BPC = cfg.NBLK // NCH           # blocks per chunk
    h_shards = [
        nc.dram_tensor(f"h_shard{k}", [CROWS, D], bf16, kind="Internal")
        for k in range(NCH)
    ]
    h_full = nc.dram_tensor(
        "h_full", [cfg.NPAD, D], bf16, kind="Internal", addr_space="Shared"
    )
    ah_shards = [
        nc.dram_tensor(f"ah_shard{k}", [CROWS, D], bf16, kind="Internal")
        for k in range(NCH)
    ]
    ah_full = nc.dram_tensor(
        "ah_full", [cfg.NPAD, D], bf16, kind="Internal", addr_space="Shared"
    )
    RG = [list(range(cfg.NC))]
    NCH_ROWS = CROWS * cfg.NC
    KD = D // 128  # k-chunks of the feature dim (=2)

    with TileKernel(nc) as tk:
        const = tk.pool("const", bufs=1)
        side = tk.pool("side", bufs=2)
        gpool = tk.pool("gpool", bufs=3)
        mpool = tk.pool("mpool", bufs=6)
        spool = tk.pool("spool", bufs=2)
        psA = tk.pool("psA", bufs=2, space="PSUM")
        psT = tk.pool("psT", bufs=2, space="PSUM")
        psH = tk.pool("psH", bufs=2, space="PSUM")
        psL = tk.pool("psL", bufs=1, space="PSUM")

        # ---------------- constants / weights in SBUF
        wi_sb = [const.tile([128, D], bf16, tag=f"wi{k}", name=f"wi_sb{k}")
                 for k in range(KD)]
        w1_sb = [const.tile([128, D], bf16, tag=f"w1{k}", name=f"w1_sb{k}")
                 for k in range(KD)]
        w2_sb = [const.tile([128, D], bf16, tag=f"w2{k}", name=f"w2_sb{k}")
                 for k in range(KD)]
        for k in range(KD):
            nc.sync.dma_start(out=wi_sb[k][:], in_=wi[k * 128 : (k + 1) * 128, :])
            nc.sync.dma_start(out=w1_sb[k][:], in_=w1[k * 128 : (k + 1) * 128, :])
            nc.sync.dma_start(out=w2_sb[k][:], in_=w2[k * 128 : (k + 1) * 128, :])
        wo_sb = const.tile([128, KD], bf16, tag="wo")
        nc.sync.dma_start(out=wo_sb[:], in_=wo[:])
        bin_sb = const.tile([1, D], bf16, tag="bin")
        nc.sync.dma_start(out=bin_sb[:], in_=bin_[:])
        bout_sb = const.tile([1, 1], bf16, tag="bout")
        nc.sync.dma_start(out=bout_sb[:], in_=bout[:])
        ones1 = const.tile([1, 128], bf16, tag="ones1")
        nc.gpsimd.memset(ones1[:], 1.0)
        iota_i = const.tile([128, 128], mybir.dt.int32, tag="iotai")
        nc.gpsimd.iota(iota_i[:], pattern=[[1, 128]], base=0, channel_multiplier=0)
        NTMAX = cfg.SUBMAX // 128
        iota_rep_i = const.tile([128, NTMAX * 128], mybir.dt.int32, tag="iotari")
        nc.gpsimd.iota(iota_rep_i[:], pattern=[[0, NTMAX], [1, 128]], base=0,
                       channel_multiplier=0)
        iota_rep = const.tile([128, NTMAX * 128], bf16, tag="iotarep")
        nc.vector.tensor_copy(out=iota_rep[:], in_=iota_rep_i[:])
        ident = const.tile([128, 128], bf16, tag="ident")
        make_identity(nc, ident[:])
        identf = const.tile([128, 128], f32, tag="identf")
        make_identity(nc, identf[:])

        xt_sb = [const.tile([128, cfg.RPC], bf16, tag=f"xt{k}", name=f"xt_sb{k}")
                 for k in range(KD)]
        for k in range(KD):
            nc.sync.dma_start(out=xt_sb[k][:], in_=xt[k * 128 : (k + 1) * 128, :])

        aht_res = const.tile([128, NBLK * D], bf16, tag="aht")
        stage = const.tile([128, NBLK], f32, tag="stage")

        relu = mybir.ActivationFunctionType.Relu

        # ---------------- phase A: H = relu(X @ W_in.T + b_in) on own shard
        for b in range(NBLK):
            ph = psA.tile([128, D], f32, tag="acc", name=f"ph_{b}")
            for k in range(KD):
                nc.tensor.matmul(
                    ph[:],
                    lhsT=xt_sb[k][:, b * 128 : (b + 1) * 128],
                    rhs=wi_sb[k][:],
                    start=(k == 0),
                    stop=False,
                )
            nc.tensor.matmul(
                ph[:], lhsT=ones1[:], rhs=bin_sb[:], start=False, stop=True
            )
            h_sb = spool.tile([128, D], bf16, tag="hsb", name=f"h_sb_{b}")
            nc.scalar.activation(out=h_sb[:], in_=ph[:], func=relu)
            bl = b % BPC
            nc.sync.dma_start(
                out=h_shards[b // BPC][bl * 128 : (bl + 1) * 128, :],
                in_=h_sb[:],
            )
            if b % BPC == BPC - 1:
                k = b // BPC
                nc.gpsimd.collective_compute(
                    "AllGather",
                    mybir.AluOpType.bypass,
                    replica_groups=RG,
                    ins=[h_shards[k][:]],
                    outs=[h_full[k * NCH_ROWS : (k + 1) * NCH_ROWS, :]],
                )

        # ---------------- SpMM via bucketed dma_gather + one-hot matmul
        def spmm_blocks(src_full, epilogue, phase):
            qrr = [0]
            for c0 in range(0, NBLK, CHUNK):
                i_sb = side.tile([128, CHUNK * IPB], i16, tag="eidx",
                                 name=f"i_sb_{phase}_{c0}")
                d_sb = side.tile([128, CHUNK * TPB], bf16, tag="edloc",
                                 name=f"d_sb_{phase}_{c0}")
                v_sb = side.tile([128, CHUNK * TPB], bf16, tag="evals",
                                 name=f"v_sb_{phase}_{c0}")
                nc.sync.dma_start(
                    out=i_sb[:], in_=eidx[:, c0 * IPB : (c0 + CHUNK) * IPB]
                )
                nc.sync.dma_start(
                    out=d_sb[:], in_=edloc[:, c0 * TPB : (c0 + CHUNK) * TPB]
                )
                nc.sync.dma_start(
                    out=v_sb[:], in_=evals[:, c0 * TPB : (c0 + CHUNK) * TPB]
                )
                for rb in range(CHUNK):
                    b = c0 + rb
                    pacc = psA.tile([128, D], f32, tag="acc",
                                    name=f"pacc_{phase}_{b}")
                    Gs = []
                    for q in range(NBUCK):
                        Tq = caps[q] // 128
                        base = q * cfg.BW
                        bsz = min(cfg.BW, cfg.NPAD - base)
                        G = gpool.tile([128, Tq * D], bf16, tag=f"G{q}",
                                       name=f"G_{phase}_{b}_{q}")
                        io = rb * IPB + IOFF[q]
                        done = 0
                        while done < caps[q]:
                            S = min(cfg.SUBMAX, caps[q] - done)
                            a = done // 128
                            nc.gpsimd.dma_gather(
                                G[:, a * D : (a + S // 128) * D].rearrange(
                                    "p (t d) -> p t d", d=D
                                ),
                                src_full[base : base + bsz, :],
                                i_sb[:, io + a * 8 : io + a * 8 + S // 16],
                                S,
                                S,
                                D,
                                queue_num=qrr[0] % nc.num_swdge_queues,
                            )
                            qrr[0] += 1
                            done += S
                        Gs.append(G)
                    for q in range(NBUCK):
                        Tq = caps[q] // 128
                        a = 0
                        while a < Tq:
                            nt = min(cfg.SUBMAX // 128, Tq - a)
                            j0 = rb * TPB + TOFF[q] + a
                            eq = mpool.tile([128, nt * 128], bf16, tag="EQ",
                                            name=f"eq_{phase}_{b}_{q}_{a}")
                            nc.vector.tensor_tensor(
                                out=eq[:],
                                in0=iota_rep[:, : nt * 128],
                                in1=d_sb[:, j0 : j0 + nt].to_broadcast(
                                    [128, nt, 128]
                                ),
                                op=mybir.AluOpType.is_equal,
                            )
                            m = mpool.tile([128, nt * 128], bf16, tag="M",
                                           name=f"m_{phase}_{b}_{q}_{a}")
                            nc.vector.tensor_tensor(
                                out=m[:],
                                in0=eq[:],
                                in1=v_sb[:, j0 : j0 + nt].to_broadcast(
                                    [128, nt, 128]
                                ),
                                op=mybir.AluOpType.mult,
                            )
                            for tq in range(nt):
                                t = TOFF[q] + a + tq
                                nc.tensor.matmul(
                                    pacc[:],
                                    lhsT=m[:, tq * 128 : (tq + 1) * 128],
                                    rhs=Gs[q][:, (a + tq) * D : (a + tq + 1) * D],
                                    start=(t == 0),
                                    stop=(t == TPB - 1),
                                )
                            a += nt
                    epilogue(b, pacc)

        # ---------------- phase B: AH = A @ H  (store shard + transposed copy)
        def epi_ah(b, pacc):
            ah_sb = spool.tile([128, D], bf16, tag="ahsb", name=f"ah_sb_{b}")
            nc.vector.tensor_copy(out=ah_sb[:], in_=pacc[:])
            bl = b % BPC
            nc.sync.dma_start(
                out=ah_shards[b // BPC][bl * 128 : (bl + 1) * 128, :],
                in_=ah_sb[:],
            )
            if b % BPC == BPC - 1:
                k = b // BPC
                nc.gpsimd.collective_compute(
                    "AllGather",
                    mybir.AluOpType.bypass,
                    replica_groups=RG,
                    ins=[ah_shards[k][:]],
                    outs=[ah_full[k * NCH_ROWS : (k + 1) * NCH_ROWS, :]],
                )
            for k in range(KD):
                pt = psT.tile([128, 128], bf16, tag="tp", name=f"ptb_{b}_{k}")
                nc.tensor.transpose(
                    out=pt[:],
                    in_=ah_sb[:, k * 128 : (k + 1) * 128],
                    identity=ident[:],
                )
                nc.vector.tensor_copy(
                    out=aht_res[:, b * D + k * 128 : b * D + (k + 1) * 128],
                    in_=pt[:],
                )

        spmm_blocks(h_full, epi_ah, "B")

        # ---------------- phase C: A2H + dense head, fused per block
        def epi_head(b, pacc):
            a2h_sb = spool.tile([128, D], bf16, tag="a2hsb", name=f"a2h_sb_{b}")
            nc.vector.tensor_copy(out=a2h_sb[:], in_=pacc[:])
            a2ht_sb = spool.tile([128, D], bf16, tag="a2ht", name=f"a2ht_sb_{b}")
            for k in range(KD):
                pt = psT.tile([128, 128], bf16, tag="tp", name=f"ptc_{b}_{k}")
                nc.tensor.transpose(
                    out=pt[:],
                    in_=a2h_sb[:, k * 128 : (k + 1) * 128],
                    identity=ident[:],
                )
                nc.vector.tensor_copy(
                    out=a2ht_sb[:, k * 128 : (k + 1) * 128], in_=pt[:]
                )
            # H2.T = relu(W1 @ AH.T + W2 @ A2H.T), by 128-row halves of h2
            h2t_sb = spool.tile([128, D], bf16, tag="h2t", name=f"h2t_sb_{b}")
            for jj in range(KD):
                ph2 = psH.tile([128, 128], f32, tag="h2", name=f"ph2_{b}_{jj}")
                for k in range(KD):
                    nc.tensor.matmul(
                        ph2[:],
                        lhsT=w1_sb[k][:, jj * 128 : (jj + 1) * 128],
                        rhs=aht_res[:, b * D + k * 128 : b * D + (k + 1) * 128],
                        start=(k == 0),
                        stop=False,
                    )
                    nc.tensor.matmul(
                        ph2[:],
                        lhsT=w2_sb[k][:, jj * 128 : (jj + 1) * 128],
                        rhs=a2ht_sb[:, k * 128 : (k + 1) * 128],
                        start=False,
                        stop=(k == KD - 1),
                    )
                nc.scalar.activation(
                    out=h2t_sb[:, jj * 128 : (jj + 1) * 128], in_=ph2[:], func=relu
                )
            # logits[r] = H2[r,:] @ W_out.T + b_out
            plg = psL.tile([128, 1], f32, tag="lg", name=f"plg_{b}")
            for jj in range(KD):
                nc.tensor.matmul(
                    plg[:],
                    lhsT=h2t_sb[:, jj * 128 : (jj + 1) * 128],
                    rhs=wo_sb[:, jj : jj + 1],
                    start=(jj == 0),
                    stop=False,
                )
            nc.tensor.matmul(
                plg[:], lhsT=ones1[:], rhs=bout_sb[:], start=False, stop=True
            )
            nc.vector.tensor_copy(out=stage[:, b : b + 1], in_=plg[:])

        spmm_blocks(ah_full, epi_head, "C")

        # ---------------- output: transpose stage [128, NBLK] -> [NBLK, 128]
        pso = psL.tile([NBLK, 128], f32, tag="outp")
        nc.tensor.transpose(out=pso[:], in_=stage[:], identity=identf[:])
        out_sb = spool.tile([NBLK, 128], f32, tag="outsb")
        nc.vector.tensor_copy(out=out_sb[:], in_=pso[:])
        nc.sync.dma_start(out=out[:], in_=out_sb[:])

    nc.compile()
    return nc


class TileKernel:
    """Small helper wrapping TileContext + pools in one ExitStack."""

    def __init__(self, nc):
        self.nc = nc
        self._ctx = ExitStack()
        self.tc = None

    def __enter__(self):
        self.tc = self._ctx.enter_context(tile.TileContext(self.nc))
        return self

    def pool(self, name, bufs, space="SBUF"):
        return self._ctx.enter_context(
            self.tc.tile_pool(name=name, bufs=bufs, space=space)
        )

    def __exit__(self, *exc):
        return self._ctx.__exit__(*exc)


# ---------------------------------------------------------------- host prep
def edge_caps(rows, cols, cfg: Cfg):
    """Per-bucket capacity = max over (core, dest block) run length, to 128."""
    rows = np.asarray(rows).astype(np.int64)
    cols = np.asarray(cols).astype(np.int64)
    core = rows // cfg.RPC
    blk = (rows % cfg.RPC) // 128
    buck = (cols % cfg.RPC) // (cfg.RPC // cfg.NBUCK)
    gid = (core * cfg.NBLK + blk) * cfg.NBUCK + buck
    cnt = np.bincount(gid, minlength=cfg.NC * cfg.NBLK * cfg.NBUCK)
    cnt = cnt.reshape(-1, cfg.NBUCK)
    caps = (-(-cnt.max(0) // 128) * 128).astype(np.int64)
    caps = np.maximum(caps, 128)
    return tuple(int(c) for c in caps)


def prep_inputs(X, rows, cols, vals, W_in, b_in, W_mp1, W_mp2, W_out, b_out,
                cfg: Cfg, caps: tuple):
    NC, RPC, NBLK, D = cfg.NC, cfg.RPC, cfg.NBLK, cfg.D
    NBUCK = cfg.NBUCK
    rows = np.asarray(rows).astype(np.int64)
    cols = np.asarray(cols).astype(np.int64)
    vals = np.asarray(vals, dtype=np.float32)
    X = np.asarray(X, dtype=np.float32)
    TPB = sum(caps) // 128
    IPB = sum(caps) // 16
    TOFF = [sum(caps[:q]) // 128 for q in range(NBUCK)]

    core = rows // RPC
    blk = (rows % RPC) // 128
    dloc = (rows % 128).astype(np.float32)
    # gathered layout: [chunk][rank][CROWS]; chunk == gather bucket
    NCH = cfg.NBUCK
    CROWS = cfg.RPC // NCH
    src_core = cols // RPC
    src_l = cols % RPC
    buck = src_l // CROWS
    lcol = src_core * CROWS + (src_l % CROWS)  # < NC*CROWS == BW

    gid = (core * NBLK + blk) * NBUCK + buck
    ngroups = NC * NBLK * NBUCK
    order = np.lexsort((cols, gid))
    counts = np.bincount(gid, minlength=ngroups)
    capv = np.array(caps, dtype=np.int64)
    assert (counts.reshape(-1, NBUCK) <= capv[None, :]).all()
    starts = np.zeros(ngroups, np.int64)
    starts[1:] = np.cumsum(counts)[:-1]
    g_sorted = gid[order]
    pos = np.arange(len(rows), dtype=np.int64) - starts[g_sorted]

    c_s = core[order]
    b_s = blk[order]
    q_s = buck[order]

    # dloc/vals sideband: edge i of its run at (p=i%128, tile=i//128)
    dloc_arr = np.zeros((NC, 128, NBLK * TPB), BF16_NP)
    vals_arr = np.zeros((NC, 128, NBLK * TPB), BF16_NP)
    toff = np.array(TOFF, dtype=np.int64)
    tcol = b_s * TPB + toff[q_s] + pos // 128
    p = pos % 128
    dloc_arr[c_s, p, tcol] = dloc[order].astype(BF16_NP)
    vals_arr[c_s, p, tcol] = vals[order].astype(BF16_NP)

    # int16 gather indices: edge i of its run at (p=i%16 (replicated x8),
    # col = run_icol0 + i//16)
    ioff = np.array([sum(caps[:q]) // 16 for q in range(NBUCK)], dtype=np.int64)
    idx_arr = np.zeros((NC, 16, NBLK * IPB), np.int16)
    SUB = cfg.SUBMAX
    jj = pos % SUB
    icol = b_s * IPB + ioff[q_s] + (pos // SUB) * (SUB // 16) + jj // 16
    ip = jj % 16
    idx_arr[c_s, ip, icol] = lcol[order].astype(np.int16)
    idx_arr = np.tile(idx_arr, (1, 8, 1))  # replicate to 128 partitions

    wi_h = np.ascontiguousarray(np.asarray(W_in, np.float32).T).astype(BF16_NP)
    w1_h = np.ascontiguousarray(np.asarray(W_mp1, np.float32).T).astype(BF16_NP)
    w2_h = np.ascontiguousarray(np.asarray(W_mp2, np.float32).T).astype(BF16_NP)
    wo_h = np.ascontiguousarray(
        np.asarray(W_out, np.float32).reshape(D // 128, 128).T
    ).astype(BF16_NP)
    bin_h = np.asarray(b_in, np.float32).reshape(1, D).astype(BF16_NP)
    bout_h = np.asarray(b_out, np.float32).reshape(1, 1).astype(BF16_NP)

    in_maps = []
    for c in range(NC):
        lo = c * RPC
        hi = min((c + 1) * RPC, cfg.N)
        xs = np.zeros((RPC, D), np.float32)
        if hi > lo:
            xs[: hi - lo] = X[lo:hi]
        xt_h = np.ascontiguousarray(xs.T).astype(BF16_NP)
        in_maps.append(
            {
                "xt": xt_h,
                "wi": wi_h,
                "w1": w1_h,
                "w2": w2_h,
                "wo": wo_h,
                "bin": bin_h,
                "bout": bout_h,
                "eidx": idx_arr[c],
                "edloc": dloc_arr[c],
                "evals": vals_arr[c],
            }
        )
    return in_maps


# ---------------------------------------------------------------- entry
_NC_CACHE = {}


def _get_nc(cfg: Cfg, caps: tuple):
    key = (cfg.NC, cfg.RPC, cfg.NBLK, cfg.BW, cfg.CHUNK, caps)
    if key not in _NC_CACHE:
        _NC_CACHE[key] = build_nc(cfg, caps)
    return _NC_CACHE[key]


def run(inputs, trace=False):
    from concourse.bass_utils import run_bass_kernel_spmd

    cfg = CFG
    caps = edge_caps(inputs["rows"], inputs["cols"], cfg)
    nc = _get_nc(cfg, caps)
    in_maps = prep_inputs(**inputs, cfg=cfg, caps=caps)
    res = run_bass_kernel_spmd(
        nc, in_maps, core_ids=list(range(cfg.NC)), trace=trace
    )
    pieces = [res.results[c]["out"].reshape(-1) for c in range(cfg.NC)]
    full = np.concatenate(pieces)[: cfg.N]
    return full.reshape(cfg.N, 1).astype(np.float32), res


def kernel(**inputs):
    out, _ = run(inputs, trace=False)
    return out


if __name__ == "__main__":
    import reference

    inp = {k: np.asarray(v) for k, v in reference.setup_inputs().items()}
    got = kernel(**inp)
    print("kernel output:", got.shape, got.dtype)
